# revision 19
# baseline (speedup 1.0000x reference)
"""Trainium2 Bass kernel: batched dense attention
   out = softmax((x_q Wq^T + bq)(x_k Wk^T + bk)^T / sqrt(E)) (x_v Wv^T + bv)

Sharding: 8 cores = 4 batches x 2 query-row halves (sequence-parallel on
Q). K/V projections are also split across the pair (each core projects
its own half of K/V) and the halves are exchanged with one in-pair
AllGather, so every core does exactly 1/8 of the total FLOPs.

Key trick: softmax attention is invariant to a permutation of the key
axis, so each core uses its own LOCAL key order [my half; partner half].
The locally projected half is written straight into the resident SBUF
K^T/V tiles (no DRAM round trip), and only the partner block of the
AllGather output is loaded back, addressed with a partition_id-derived
dynamic DRAM offset.

Device scheme (matmul operands bf16 — measured faster than fp16 on HW
for 512-wide moving operands — with fp32 PSUM accumulation):
  - scores are computed TRANSPOSED (keys on partitions) so the exp'd
    probabilities feed the P@V matmul as the stationary operand with no
    on-device transposes (host pre-transposes x^T / W^T and pre-casts).
  - softmax without max-subtraction (scores ~ N(0,1) at this scale;
    exp is safe in fp32): Z = sum_k exp(s) accumulated via a
    ones-vector matmul; 1/Z applied during the PSUM->SBUF output copy.
  - attention is split into two passes so TensorE never waits on the
    exchange: [2a] every query chunk vs the OWN key half (with its Q
    projection computed just-in-time), partial P@V sums drained to DRAM
    in fp16; [2b] every query chunk vs the PARTNER half, combined with
    the 2a partials and normalized by the full softmax denominator.
    Partner K/V is first touched ~160us after the gather lands.
  - DMA completion semaphores rotate GLOBALLY across queues: any DMA
    whose rotation slot lands after a still-pending collective blocks
    until that collective completes. So every latency-critical load
    (projection feeds, wq, the first 4 query-chunk feeds) is scheduled
    BEFORE the gather, the gather + partner loads are pinned to
    sched-0.25 via tile_wait_until, and everything issued after them
    (late xq feeds, partial writes, readbacks, stores) has >100us of
    deadline slack.
  - attention consumes key tiles with the P@V matmuls trailing the
    score/exp pipeline by one key-tile so PSUM handoffs stay off the
    TensorE critical path.
  - bv folded to the end (softmax rows sum to 1 => P@(V+bv) = P@V+bv),
    and skipped entirely when bv == 0 (a per-bias-pattern kernel
    variant is compiled).
"""

import numpy as np

import concourse.bacc as bacc
import concourse.bass as bass
import concourse.mybir as mybir
import concourse.tile as tile
from concourse.bass_utils import run_bass_kernel_spmd

B, S, E = 4, 4096, 1024
N_CORES = 8
HQ = 2              # halves per batch
SQ = S // HQ        # 2048 rows per core (own query rows / own K,V rows)
P = 128
ET = E // P         # 8 embed tiles
SKT = S // P        # 32 key tiles (local order: 0-15 own, 16-31 partner)
SKH = SKT // 2      # 16 key tiles per half
NQ = 256            # attention query-chunk (2 psum out subtiles)
NQT = SQ // NQ      # 8 chunks per core
CH = 512            # projection column chunk
HCH = SQ // CH      # 4 chunks for the K/V half projections
INV_SCALE = 1.0 / float(E) ** 0.5
GROUPS = [[0, 1], [2, 3], [4, 5], [6, 7]]
QPRE = 4            # query-chunk feeds preloaded ahead of the gather

BF16 = mybir.dt.bfloat16
F16 = mybir.dt.float16   # only for the DMA-staged P@V partials
F32 = mybir.dt.float32
AF = mybir.ActivationFunctionType

_CACHE = {}


def _emit(nc, tc, dram, with_bv):
    xqT, xkT, xvT, wqT, wkT, wvT, bqr, bkr, bvb, out = dram

    # partner-block row offset into the AllGather output (dynamic DMA
    # offsets must live in a register on the issuing engine; the partner
    # loads are issued from the gpsimd queue)
    pid_g = nc.gpsimd.partition_id()
    off_g = (1 - (pid_g & 1)) * (2 * E)

    with (
        tc.tile_pool(name="consts", bufs=1) as cpool,
        tc.tile_pool(name="kv", bufs=1) as kvpool,
        tc.tile_pool(name="ccd", bufs=1, space="DRAM") as dpool,
    ):
        bq_sb = cpool.tile([P, ET], F32)
        nc.gpsimd.dma_start(bq_sb[:], bqr[:])
        bk_sb = cpool.tile([P, ET], F32)
        nc.gpsimd.dma_start(bk_sb[:], bkr[:])
        if with_bv:
            bv_sb = cpool.tile([P, E], F32)
            nc.gpsimd.dma_start(bv_sb[:], bvb[:])
        ones = cpool.tile([P, 1], F32)
        nc.gpsimd.memset(ones[:], 1.0)

        # resident OWN K^T half [E, SQ] and V half [SQ, E] (bf16, 8 MB)
        kTo = [kvpool.tile([P, SQ], BF16, tag=f"kTo{et}", name=f"kTo{et}")
               for et in range(ET)]
        vN = [kvpool.tile([P, E], BF16, tag=f"v{sk}", name=f"v{sk}")
              for sk in range(SKH)]

        # one combined K+V exchange buffer: rows [0,E) = K^T half
        # ([E, SQ], natural layout), rows [E,2E) = V half packed two
        # key-tiles per 128-row band ([128, 2E] per band)
        cb_ib = dpool.tile([2 * E, SQ], BF16, tag="cbi", name="cbi")
        cb_ob = dpool.tile([2 * 2 * E, SQ], BF16, tag="cbo", name="cbo")

        # fp16 own-half P@V partial sums, staged through DRAM between
        # passes 2a and 2b (SBUF has no room for 4MB of partials)
        apart = [dpool.tile([P, E], F16, tag=f"ap{r}", name=f"ap{r}")
                 for r in range(2 * NQT)]

        # ---- Phase 1: half K / half V projections ----
        with (
            tc.tile_pool(name="wkv", bufs=16) as wpool,
            tc.tile_pool(name="xk", bufs=14) as xkpool,
            tc.tile_pool(name="xv", bufs=13) as xvpool,
            tc.tile_pool(name="psk", bufs=3, space="PSUM") as psk,
            tc.tile_pool(name="psv", bufs=4, space="PSUM") as psv,
        ):
            wk_sb = [wpool.tile([P, E], BF16, tag="w", name="wk")
                     for _ in range(ET)]
            for dt in range(ET):
                nc.scalar.dma_start(wk_sb[dt][:], wkT[dt * P:(dt + 1) * P, :])

            def feed(src_t, ch, pool):
                cs = slice(ch * CH, (ch + 1) * CH)
                ts = [pool.tile([P, CH], BF16, tag="x", name="xf")
                      for _ in range(ET)]
                for dt in range(ET):
                    nc.sync.dma_start(ts[dt][:], src_t[dt * P:(dt + 1) * P, cs])
                return ts

            feeds = {("k", 0): feed(xkT, 0, xkpool),
                     ("v", 0): feed(xvT, 0, xvpool)}
            wv_sb = [wpool.tile([P, E], BF16, tag="w", name="wv")
                     for _ in range(ET)]
            for dt in range(ET):
                nc.scalar.dma_start(wv_sb[dt][:], wvT[dt * P:(dt + 1) * P, :])
            for ch in range(HCH):
                cs = slice(ch * CH, (ch + 1) * CH)
                # own-half k^T[e, s-chunk] = sum_d WkT[d, e] * xkT[d, s-chunk]
                # written straight into kT columns [0, SQ)
                xs = feeds.pop(("k", ch))
                if ch + 1 < HCH:
                    feeds[("k", ch + 1)] = feed(xkT, ch + 1, xkpool)
                for et in range(ET):
                    ps = psk.tile([P, CH], F32, tag="pk")
                    for dt in range(ET):
                        nc.tensor.matmul(
                            ps[:], wk_sb[dt][:, et * P:(et + 1) * P], xs[dt][:],
                            start=(dt == 0), stop=(dt == ET - 1))
                    nc.vector.tensor_scalar_add(
                        kTo[et][:, cs], ps[:], bk_sb[:, et:et + 1])
                    nc.scalar.dma_start(cb_ib[et * P:(et + 1) * P, cs],
                                        kTo[et][:, cs])

                # own-half v[s-chunk, e] = sum_d xvT[d, s-chunk] * WvT[d, e]
                xv = feeds.pop(("v", ch))
                if ch + 1 < HCH:
                    feeds[("v", ch + 1)] = feed(xvT, ch + 1, xvpool)
                for si in range(CH // P):
                    sk = ch * (CH // P) + si
                    ph = [psv.tile([P, 512], F32, tag="pv", name="pv")
                          for _ in range(2)]
                    for dt in range(ET):
                        for nh in range(2):
                            nc.tensor.matmul(
                                ph[nh][:],
                                xv[dt][:, si * P:(si + 1) * P],
                                wv_sb[dt][:, nh * 512:(nh + 1) * 512],
                                start=(dt == 0), stop=(dt == ET - 1))
                    for nh in range(2):
                        nc.vector.tensor_copy(
                            vN[sk][:, nh * 512:(nh + 1) * 512], ph[nh][:])
                    # V pack: row band E + (sk//2)*P, col block (sk%2)*E
                    nc.scalar.dma_start(
                        cb_ib[E + (sk // 2) * P:E + (sk // 2 + 1) * P,
                              (sk % 2) * E:(sk % 2 + 1) * E],
                        vN[sk][:, :])

        # ---- long-lived attention state (fits in the space the ----
        # ---- phase-1 pools released)                            ----
        kvp = tc.alloc_tile_pool(name="kvp", bufs=1)
        kTp = kvp.tile([P, ET * SQ], BF16, tag="kTp", name="kTp")
        vNp = kvp.tile([P, SKH * E], BF16, tag="vNp", name="vNp")
        qtpool = tc.alloc_tile_pool(name="qt", bufs=1)
        qt = [[qtpool.tile([P, NQ], BF16, tag=f"qt{qc}_{et}",
                           name=f"qt{qc}_{et}")
               for et in range(ET)] for qc in range(NQT)]
        accpool = tc.alloc_tile_pool(name="acc", bufs=1)
        acc = [accpool.tile([P, NQ], F32, tag=f"acc{qc}", name=f"acc{qc}")
               for qc in range(NQT)]
        # wq + xq feeds: released after 2a (2b needs the space).
        # Emitted before the pinned gather so the scheduler gives these
        # loads rotation slots ahead of it (they land by ~120us).
        wqx = tc.alloc_tile_pool(name="wqx", bufs=1)
        wq_sb = [wqx.tile([P, E], BF16, tag="w", name="wq", bufs=8)
                 for _ in range(ET)]
        for dt in range(ET):
            nc.scalar.dma_start(wq_sb[dt][:], wqT[dt * P:(dt + 1) * P, :])

        def qfeed(qc):
            ts = [wqx.tile([P, NQ], BF16, tag="qx", name="xq", bufs=8 * QPRE)
                  for _ in range(ET)]
            for dt in range(ET):
                nc.sync.dma_start(
                    ts[dt][:],
                    xqT[dt * P:(dt + 1) * P, qc * NQ:(qc + 1) * NQ])
            return ts

        xq_pre = {qc: qfeed(qc) for qc in range(QPRE)}

        def kT(et, sk):
            if sk < SKH:
                return kTo[et][:, sk * P:(sk + 1) * P]
            lo = sk - SKH
            return kTp[:, et * SQ + lo * P:et * SQ + (lo + 1) * P]

        def vT(sk, nh):
            if sk < SKH:
                return vN[sk][:, nh * 512:(nh + 1) * 512]
            lo = sk - SKH
            return vNp[:, lo * E + nh * 512:lo * E + (nh + 1) * 512]

        # ---- the exchange: ONE in-pair AllGather + 2 partner loads ----
        # pinned to sched-0.25: after every latency-critical feed above
        # (so the pending collective can't poison their completion
        # semaphores in the global rotation), but ~250us of real time
        # before 2b first touches partner K/V.
        with tc.tile_wait_until(0.25):
            nc.gpsimd.collective_compute(
                "AllGather", mybir.AluOpType.bypass, replica_groups=GROUPS,
                ins=[cb_ib.opt()], outs=[cb_ob.opt()])
            # K: cb_ob rows [off+et*P+p, s] -> kTp[p, et*SQ + s]
            nc.gpsimd.dma_start(
                kTp.rearrange("p (et s) -> p et s", et=ET),
                cb_ob[bass.ds(off_g, E), :].rearrange(
                    "(et p) s -> p et s", et=ET))
            # V: cb_ob rows [off+E+skh*P+p, two*E+e] -> vNp[p, (2*skh+two)*E+e]
            nc.gpsimd.dma_start(
                vNp.rearrange("p (skh two e) -> p skh two e", skh=SKH // 2,
                              two=2),
                cb_ob[bass.ds(off_g + E, E), :].rearrange(
                    "(skh p) (two e) -> p skh two e", skh=SKH // 2, two=2))

        # ---------------- Phase 2: attention ----------------
        def attn_pass(qc, base, first_pass, pools):
            pss, pso, eppool = pools
            po = [pso.tile([P, E], F32, tag="po", name="po")
                  for _ in range(2)]

            def emit_pv(ep, sk, first, last):
                for j in range(2):
                    lhs = ep[:, j * P:(j + 1) * P]
                    for nh in range(2):
                        nc.tensor.matmul(
                            po[j][:, nh * 512:(nh + 1) * 512], lhs,
                            vT(sk, nh), start=first, stop=last)

            # scores/exp run one key-tile ahead of the P@V accumulation
            prev = None
            for idx in range(SKH):
                sk = base + idx
                ps = pss.tile([P, NQ], F32, tag="ps")
                for et in range(ET):
                    nc.tensor.matmul(
                        ps[:], kT(et, sk), qt[qc][et][:],
                        start=(et == 0), stop=(et == ET - 1))
                ep = eppool.tile([P, NQ], BF16, tag="ep")
                nc.scalar.activation(ep[:], ps[:], AF.Exp, scale=INV_SCALE)
                if first_pass and idx == 0:
                    nc.vector.tensor_copy(acc[qc][:], ep[:])
                else:
                    nc.vector.tensor_add(acc[qc][:], acc[qc][:], ep[:])
                if prev is not None:
                    emit_pv(*prev, first=(idx == 1), last=False)
                prev = (ep, sk)
            emit_pv(*prev, first=False, last=True)
            return po

        # ---- 2a: JIT Q projection + own key half; drain partials ----
        with (
            tc.tile_pool(name="ep", bufs=4) as eppool,
            tc.tile_pool(name="ast", bufs=2) as astpool,
            tc.tile_pool(name="pss", bufs=2, space="PSUM") as pss,
            tc.tile_pool(name="pso", bufs=2, space="PSUM") as pso,
        ):
            for qc in range(NQT):
                xq = xq_pre.pop(qc)
                if qc + QPRE < NQT:
                    xq_pre[qc + QPRE] = qfeed(qc + QPRE)
                for et in range(ET):
                    ps = pss.tile([P, NQ], F32, tag="ps")
                    for dt in range(ET):
                        nc.tensor.matmul(
                            ps[:], wq_sb[dt][:, et * P:(et + 1) * P], xq[dt][:],
                            start=(dt == 0), stop=(dt == ET - 1))
                    nc.vector.tensor_scalar_add(
                        qt[qc][et][:], ps[:], bq_sb[:, et:et + 1])
                po = attn_pass(qc, 0, True, (pss, pso, eppool))
                for j in range(2):
                    ast = astpool.tile([P, E], F16, tag="ast", name="ast")
                    nc.vector.tensor_copy(ast[:], po[j][:])
                    nc.gpsimd.dma_start(apart[qc * 2 + j][:], ast[:])
        wqx.release()

        # ---- 2b: partner key half; combine partials + normalize ----
        with (
            tc.tile_pool(name="ep2", bufs=4) as eppool,
            tc.tile_pool(name="ard", bufs=4) as ardpool,
            tc.tile_pool(name="fin", bufs=3) as finpool,
            tc.tile_pool(name="ps2", bufs=2, space="PSUM") as pss,
            tc.tile_pool(name="po2", bufs=2, space="PSUM") as pso,
            tc.tile_pool(name="psz", bufs=2, space="PSUM") as psz,
        ):
            def aread(qc):
                ts = [ardpool.tile([P, E], F16, tag="ard", name="ard")
                      for _ in range(2)]
                for j in range(2):
                    nc.sync.dma_start(ts[j][:], apart[qc * 2 + j][:])
                return ts

            nxt = aread(0)
            for qc in range(NQT):
                ard = nxt
                if qc + 1 < NQT:
                    nxt = aread(qc + 1)
                po = attn_pass(qc, SKH, False, (pss, pso, eppool))

                pz = [psz.tile([P, 1], F32, tag="pz", name="pz")
                      for _ in range(2)]
                for j in range(2):
                    nc.tensor.matmul(pz[j][:], acc[qc][:, j * P:(j + 1) * P],
                                     ones[:], start=True, stop=True)

                for j in range(2):
                    zi = finpool.tile([P, 1], F32, tag="zi", name="zi")
                    nc.vector.reciprocal(zi[:], pz[j][:])
                    r = qc * 2 + j
                    # half-column pipelining: vector add (psum+partial),
                    # scalar copy applying 1/Z, DMA out
                    for oh in range(2):
                        cs = slice(oh * 512, (oh + 1) * 512)
                        ob = finpool.tile([P, 512], F32, tag="ob", name="ob")
                        nc.vector.tensor_add(ob[:], po[j][:, cs],
                                             ard[j][:, cs])
                        ob2 = finpool.tile([P, 512], F32, tag="ob2",
                                           name="ob2")
                        nc.scalar.activation(ob2[:], ob[:], AF.Copy,
                                             scale=zi[:])
                        if with_bv:
                            nc.vector.tensor_add(ob2[:], ob2[:],
                                                 bv_sb[:, cs])
                        nc.gpsimd.dma_start(out[r * P:(r + 1) * P, cs],
                                            ob2[:])
        accpool.release()
        qtpool.release()
        kvp.release()


def _build(with_bv):
    key = ("nc", with_bv)
    if key in _CACHE:
        return _CACHE[key]
    nc = bacc.Bacc("TRN2", target_bir_lowering=False, debug=False,
                   num_devices=N_CORES)
    dram = (
        nc.dram_tensor("xqT", [E, SQ], BF16, kind="ExternalInput"),
        nc.dram_tensor("xkT", [E, SQ], BF16, kind="ExternalInput"),
        nc.dram_tensor("xvT", [E, SQ], BF16, kind="ExternalInput"),
        nc.dram_tensor("wqT", [E, E], BF16, kind="ExternalInput"),
        nc.dram_tensor("wkT", [E, E], BF16, kind="ExternalInput"),
        nc.dram_tensor("wvT", [E, E], BF16, kind="ExternalInput"),
        nc.dram_tensor("bqr", [P, ET], F32, kind="ExternalInput"),
        nc.dram_tensor("bkr", [P, ET], F32, kind="ExternalInput"),
        nc.dram_tensor("bvb", [P, E], F32, kind="ExternalInput"),
        nc.dram_tensor("out", [SQ, E], F32, kind="ExternalOutput"),
    )
    with tile.TileContext(nc) as tc:
        _emit(nc, tc, dram, with_bv)
    nc.compile()
    _CACHE[key] = nc
    return nc


def _prep_in_maps(query, key, value, Wq, bq, Wk, bk, Wv, bv):
    import ml_dtypes
    bf = ml_dtypes.bfloat16
    wqT = np.ascontiguousarray(np.asarray(Wq, np.float32).T.astype(bf))
    wkT = np.ascontiguousarray(np.asarray(Wk, np.float32).T.astype(bf))
    wvT = np.ascontiguousarray(np.asarray(Wv, np.float32).T.astype(bf))
    bqr = np.ascontiguousarray(np.asarray(bq, np.float32).reshape(ET, P).T)
    bkr = np.ascontiguousarray(np.asarray(bk, np.float32).reshape(ET, P).T)
    bvb = np.ascontiguousarray(
        np.broadcast_to(np.asarray(bv, np.float32), (P, E)))
    query = np.asarray(query, np.float32)
    key = np.asarray(key, np.float32)
    value = np.asarray(value, np.float32)
    in_maps = []
    for c in range(N_CORES):
        b, h = divmod(c, HQ)
        sl = slice(h * SQ, (h + 1) * SQ)
        in_maps.append({
            "xqT": np.ascontiguousarray(query[b, sl, :].T.astype(bf)),
            "xkT": np.ascontiguousarray(key[b, sl, :].T.astype(bf)),
            "xvT": np.ascontiguousarray(value[b, sl, :].T.astype(bf)),
            "wqT": wqT, "wkT": wkT, "wvT": wvT,
            "bqr": bqr, "bkr": bkr, "bvb": bvb,
        })
    return in_maps


def kernel(query, key, value, Wq, bq, Wk, bk, Wv, bv, _run_kwargs=None):
    with_bv = bool(np.any(np.asarray(bv, np.float32)))
    nc = _build(with_bv)
    in_maps = _prep_in_maps(query, key, value, Wq, bq, Wk, bk, Wv, bv)
    res = run_bass_kernel_spmd(nc, in_maps, core_ids=list(range(N_CORES)),
                               **(_run_kwargs or {}))
    out = np.empty((B, S, E), np.float32)
    for c in range(N_CORES):
        b, h = divmod(c, HQ)
        out[b, h * SQ:(h + 1) * SQ, :] = res.results[c]["out"]
    if _run_kwargs:
        _CACHE["last_results"] = res
    return out


# revision 21
# speedup vs baseline: 1.2083x; 1.2083x over previous
"""Trainium2 Bass kernel: batched dense attention
   out = softmax((x_q Wq^T + bq)(x_k Wk^T + bk)^T / sqrt(E)) (x_v Wv^T + bv)

Sharding: 8 cores = 4 batches x 2 query-row halves (sequence-parallel on
Q). K/V projections are also split across the pair (each core projects
its own half of K/V) and the halves are exchanged with one in-pair
AllGather, so every core does exactly 1/8 of the total FLOPs.

Key trick: softmax attention is invariant to a permutation of the key
axis, so each core uses its own LOCAL key order [my half; partner half].
The locally projected half is written straight into the resident SBUF
K^T/V tiles (no DRAM round trip), and only the partner block of the
AllGather output is loaded back, addressed with a partition_id-derived
dynamic DRAM offset.

Device scheme (matmul operands bf16 — measured faster than fp16 on HW
for 512-wide moving operands — with fp32 PSUM accumulation):
  - scores are computed TRANSPOSED (keys on partitions) so the exp'd
    probabilities feed the P@V matmul as the stationary operand with no
    on-device transposes (host pre-transposes x^T / W^T and pre-casts).
  - softmax without max-subtraction (scores ~ N(0,1) at this scale;
    exp is safe in fp32): Z = sum_k exp(s) accumulated via a
    ones-vector matmul; 1/Z applied during the PSUM->SBUF output copy.
  - attention is split into two passes so TensorE never waits on the
    exchange: [2a] every query chunk vs the OWN key half (with its Q
    projection computed just-in-time), partial P@V sums drained to DRAM
    in fp16; [2b] every query chunk vs the PARTNER half, combined with
    the 2a partials and normalized by the full softmax denominator.
    Partner K/V is first touched ~160us after the gather lands.
  - DMA completion semaphores rotate GLOBALLY across queues: any DMA
    whose rotation slot lands after a still-pending collective blocks
    until that collective completes. So every latency-critical load
    (projection feeds, wq, the first 4 query-chunk feeds) is scheduled
    BEFORE the gather, the gather + partner loads are pinned to
    sched-0.25 via tile_wait_until, and everything issued after them
    (late xq feeds, partial writes, readbacks, stores) has >100us of
    deadline slack.
  - attention consumes key tiles with the P@V matmuls trailing the
    score/exp pipeline by one key-tile so PSUM handoffs stay off the
    TensorE critical path.
  - bv folded to the end (softmax rows sum to 1 => P@(V+bv) = P@V+bv),
    and skipped entirely when bv == 0 (a per-bias-pattern kernel
    variant is compiled).
"""

import numpy as np

import concourse.bacc as bacc
import concourse.bass as bass
import concourse.mybir as mybir
import concourse.tile as tile
from concourse.bass_utils import run_bass_kernel_spmd

B, S, E = 4, 4096, 1024
N_CORES = 8
HQ = 2              # halves per batch
SQ = S // HQ        # 2048 rows per core (own query rows / own K,V rows)
P = 128
ET = E // P         # 8 embed tiles
SKT = S // P        # 32 key tiles (local order: 0-15 own, 16-31 partner)
SKH = SKT // 2      # 16 key tiles per half
NQ = 256            # attention query-chunk (2 psum out subtiles)
NQT = SQ // NQ      # 8 chunks per core
CH = 512            # projection column chunk
HCH = SQ // CH      # 4 chunks for the K/V half projections
INV_SCALE = 1.0 / float(E) ** 0.5
GROUPS = [[0, 1], [2, 3], [4, 5], [6, 7]]
QPRE = 4            # query-chunk feeds preloaded ahead of the gather

BF16 = mybir.dt.bfloat16
F16 = mybir.dt.float16   # only for the DMA-staged P@V partials
F32 = mybir.dt.float32
AF = mybir.ActivationFunctionType

_CACHE = {}


def _emit(nc, tc, dram, with_bv):
    xqT, xkT, xvT, wqT, wkT, wvT, bqr, bkr, bvb, out = dram

    # partner-block row offset into the AllGather output (dynamic DMA
    # offsets must live in a register on the issuing engine; the partner
    # loads are issued from the gpsimd queue)
    pid_g = nc.gpsimd.partition_id()
    off_g = (1 - (pid_g & 1)) * (2 * E)

    with (
        tc.tile_pool(name="consts", bufs=1) as cpool,
        tc.tile_pool(name="kv", bufs=1) as kvpool,
        tc.tile_pool(name="ccd", bufs=1, space="DRAM") as dpool,
    ):
        bq_sb = cpool.tile([P, ET], F32)
        nc.gpsimd.dma_start(bq_sb[:], bqr[:])
        bk_sb = cpool.tile([P, ET], F32)
        nc.gpsimd.dma_start(bk_sb[:], bkr[:])
        if with_bv:
            bv_sb = cpool.tile([P, E], F32)
            nc.gpsimd.dma_start(bv_sb[:], bvb[:])
        ones = cpool.tile([P, 1], F32)
        nc.gpsimd.memset(ones[:], 1.0)

        # resident OWN K^T half [E, SQ] and V half [SQ, E] (bf16, 8 MB)
        kTo = [kvpool.tile([P, SQ], BF16, tag=f"kTo{et}", name=f"kTo{et}")
               for et in range(ET)]
        vN = [kvpool.tile([P, E], BF16, tag=f"v{sk}", name=f"v{sk}")
              for sk in range(SKH)]

        # one combined K+V exchange buffer: rows [0,E) = K^T half
        # ([E, SQ], natural layout), rows [E,2E) = V half packed two
        # key-tiles per 128-row band ([128, 2E] per band)
        cb_ib = dpool.tile([2 * E, SQ], BF16, tag="cbi", name="cbi")
        cb_ob = dpool.tile([2 * 2 * E, SQ], BF16, tag="cbo", name="cbo")

        # fp16 own-half P@V partial sums, staged through DRAM between
        # passes 2a and 2b (SBUF has no room for 4MB of partials)
        apart = [dpool.tile([P, E], F16, tag=f"ap{r}", name=f"ap{r}")
                 for r in range(2 * NQT)]

        # ---- Phase 1: half K / half V projections ----
        with (
            tc.tile_pool(name="wkv", bufs=16) as wpool,
            tc.tile_pool(name="xk", bufs=14) as xkpool,
            tc.tile_pool(name="xv", bufs=13) as xvpool,
            tc.tile_pool(name="psk", bufs=3, space="PSUM") as psk,
            tc.tile_pool(name="psv", bufs=4, space="PSUM") as psv,
        ):
            wk_sb = [wpool.tile([P, E], BF16, tag="w", name="wk")
                     for _ in range(ET)]
            for dt in range(ET):
                nc.scalar.dma_start(wk_sb[dt][:], wkT[dt * P:(dt + 1) * P, :])

            def feed(src_t, ch, pool):
                cs = slice(ch * CH, (ch + 1) * CH)
                ts = [pool.tile([P, CH], BF16, tag="x", name="xf")
                      for _ in range(ET)]
                for dt in range(ET):
                    nc.sync.dma_start(ts[dt][:], src_t[dt * P:(dt + 1) * P, cs])
                return ts

            feeds = {("k", 0): feed(xkT, 0, xkpool),
                     ("v", 0): feed(xvT, 0, xvpool)}
            wv_sb = [wpool.tile([P, E], BF16, tag="w", name="wv")
                     for _ in range(ET)]
            for dt in range(ET):
                nc.scalar.dma_start(wv_sb[dt][:], wvT[dt * P:(dt + 1) * P, :])
            for ch in range(HCH):
                cs = slice(ch * CH, (ch + 1) * CH)
                # own-half k^T[e, s-chunk] = sum_d WkT[d, e] * xkT[d, s-chunk]
                # written straight into kT columns [0, SQ)
                xs = feeds.pop(("k", ch))
                if ch + 1 < HCH:
                    feeds[("k", ch + 1)] = feed(xkT, ch + 1, xkpool)
                for et in range(ET):
                    ps = psk.tile([P, CH], F32, tag="pk")
                    for dt in range(ET):
                        nc.tensor.matmul(
                            ps[:], wk_sb[dt][:, et * P:(et + 1) * P], xs[dt][:],
                            start=(dt == 0), stop=(dt == ET - 1))
                    nc.vector.tensor_scalar_add(
                        kTo[et][:, cs], ps[:], bk_sb[:, et:et + 1])
                    nc.scalar.dma_start(cb_ib[et * P:(et + 1) * P, cs],
                                        kTo[et][:, cs])

                # own-half v[s-chunk, e] = sum_d xvT[d, s-chunk] * WvT[d, e]
                xv = feeds.pop(("v", ch))
                if ch + 1 < HCH:
                    feeds[("v", ch + 1)] = feed(xvT, ch + 1, xvpool)
                for si in range(CH // P):
                    sk = ch * (CH // P) + si
                    ph = [psv.tile([P, 512], F32, tag="pv", name="pv")
                          for _ in range(2)]
                    for dt in range(ET):
                        for nh in range(2):
                            nc.tensor.matmul(
                                ph[nh][:],
                                xv[dt][:, si * P:(si + 1) * P],
                                wv_sb[dt][:, nh * 512:(nh + 1) * 512],
                                start=(dt == 0), stop=(dt == ET - 1))
                    for nh in range(2):
                        nc.vector.tensor_copy(
                            vN[sk][:, nh * 512:(nh + 1) * 512], ph[nh][:])
                    # V pack: row band E + (sk//2)*P, col block (sk%2)*E
                    nc.scalar.dma_start(
                        cb_ib[E + (sk // 2) * P:E + (sk // 2 + 1) * P,
                              (sk % 2) * E:(sk % 2 + 1) * E],
                        vN[sk][:, :])

        # ---- long-lived attention state (fits in the space the ----
        # ---- phase-1 pools released)                            ----
        kvp = tc.alloc_tile_pool(name="kvp", bufs=1)
        kTp = kvp.tile([P, ET * SQ], BF16, tag="kTp", name="kTp")
        vNp = kvp.tile([P, SKH * E], BF16, tag="vNp", name="vNp")
        qtpool = tc.alloc_tile_pool(name="qt", bufs=1)
        qt = [[qtpool.tile([P, NQ], BF16, tag=f"qt{qc}_{et}",
                           name=f"qt{qc}_{et}")
               for et in range(ET)] for qc in range(NQT)]
        accpool = tc.alloc_tile_pool(name="acc", bufs=1)
        acc = [accpool.tile([P, NQ], F32, tag=f"acc{qc}", name=f"acc{qc}")
               for qc in range(NQT)]
        # wq + xq feeds: released after 2a (2b needs the space).
        # Emitted before the pinned gather so the scheduler gives these
        # loads rotation slots ahead of it (they land by ~120us).
        wqx = tc.alloc_tile_pool(name="wqx", bufs=1)
        wq_sb = [wqx.tile([P, E], BF16, tag="w", name="wq", bufs=8)
                 for _ in range(ET)]
        for dt in range(ET):
            nc.scalar.dma_start(wq_sb[dt][:], wqT[dt * P:(dt + 1) * P, :])

        def qfeed(qc):
            ts = [wqx.tile([P, NQ], BF16, tag="qx", name="xq", bufs=8 * QPRE)
                  for _ in range(ET)]
            for dt in range(ET):
                nc.sync.dma_start(
                    ts[dt][:],
                    xqT[dt * P:(dt + 1) * P, qc * NQ:(qc + 1) * NQ])
            return ts

        xq_pre = {qc: qfeed(qc) for qc in range(QPRE)}

        def kT(et, sk):
            if sk < SKH:
                return kTo[et][:, sk * P:(sk + 1) * P]
            lo = sk - SKH
            return kTp[:, et * SQ + lo * P:et * SQ + (lo + 1) * P]

        def vT(sk, nh):
            if sk < SKH:
                return vN[sk][:, nh * 512:(nh + 1) * 512]
            lo = sk - SKH
            return vNp[:, lo * E + nh * 512:lo * E + (nh + 1) * 512]

        # ---- the exchange: ONE in-pair AllGather + 2 partner loads ----
        # pinned to sched-0.25: after every latency-critical feed above
        # (so the pending collective can't poison their completion
        # semaphores in the global rotation), but ~250us of real time
        # before 2b first touches partner K/V.
        with tc.tile_wait_until(0.25):
            nc.gpsimd.collective_compute(
                "AllGather", mybir.AluOpType.bypass, replica_groups=GROUPS,
                ins=[cb_ib.opt()], outs=[cb_ob.opt()])
            # K: cb_ob rows [off+et*P+p, s] -> kTp[p, et*SQ + s]
            nc.gpsimd.dma_start(
                kTp.rearrange("p (et s) -> p et s", et=ET),
                cb_ob[bass.ds(off_g, E), :].rearrange(
                    "(et p) s -> p et s", et=ET))
            # V: cb_ob rows [off+E+skh*P+p, two*E+e] -> vNp[p, (2*skh+two)*E+e]
            nc.gpsimd.dma_start(
                vNp.rearrange("p (skh two e) -> p skh two e", skh=SKH // 2,
                              two=2),
                cb_ob[bass.ds(off_g + E, E), :].rearrange(
                    "(skh p) (two e) -> p skh two e", skh=SKH // 2, two=2))

        # ---------------- Phase 2: attention ----------------
        def attn_pass(qc, base, first_pass, pools):
            pss, pso, eppool = pools
            po = [pso.tile([P, E], F32, tag="po", name="po")
                  for _ in range(2)]

            def emit_pv(ep, sk, first, last):
                for j in range(2):
                    lhs = ep[:, j * P:(j + 1) * P]
                    for nh in range(2):
                        nc.tensor.matmul(
                            po[j][:, nh * 512:(nh + 1) * 512], lhs,
                            vT(sk, nh), start=first, stop=last)

            # scores/exp run one key-tile ahead of the P@V accumulation
            prev = None
            for idx in range(SKH):
                sk = base + idx
                ps = pss.tile([P, NQ], F32, tag="ps")
                for et in range(ET):
                    nc.tensor.matmul(
                        ps[:], kT(et, sk), qt[qc][et][:],
                        start=(et == 0), stop=(et == ET - 1))
                ep = eppool.tile([P, NQ], BF16, tag="ep")
                nc.scalar.activation(ep[:], ps[:], AF.Exp, scale=INV_SCALE)
                if first_pass and idx == 0:
                    nc.vector.tensor_copy(acc[qc][:], ep[:])
                else:
                    nc.vector.tensor_add(acc[qc][:], acc[qc][:], ep[:])
                if prev is not None:
                    emit_pv(*prev, first=(idx == 1), last=False)
                prev = (ep, sk)
            emit_pv(*prev, first=False, last=True)
            return po

        # ---- 2a: JIT Q projection + own key half; drain partials ----
        with (
            tc.tile_pool(name="ep", bufs=4) as eppool,
            tc.tile_pool(name="ast", bufs=2) as astpool,
            tc.tile_pool(name="pss", bufs=2, space="PSUM") as pss,
            tc.tile_pool(name="pso", bufs=2, space="PSUM") as pso,
        ):
            for qc in range(NQT):
                xq = xq_pre.pop(qc)
                if qc + QPRE < NQT:
                    xq_pre[qc + QPRE] = qfeed(qc + QPRE)
                for et in range(ET):
                    ps = pss.tile([P, NQ], F32, tag="ps")
                    for dt in range(ET):
                        nc.tensor.matmul(
                            ps[:], wq_sb[dt][:, et * P:(et + 1) * P], xq[dt][:],
                            start=(dt == 0), stop=(dt == ET - 1))
                    nc.vector.tensor_scalar_add(
                        qt[qc][et][:], ps[:], bq_sb[:, et:et + 1])
                po = attn_pass(qc, 0, True, (pss, pso, eppool))
                # partial writes go on the scalar queue: the gather +
                # partner loads own the gpsimd queue, and a queue-mate
                # emitted after them could delay their issue past 2a
                for j in range(2):
                    ast = astpool.tile([P, E], F16, tag="ast", name="ast")
                    nc.vector.tensor_copy(ast[:], po[j][:])
                    nc.scalar.dma_start(apart[qc * 2 + j][:], ast[:])
        wqx.release()

        # ---- 2b: partner key half; combine partials + normalize ----
        with (
            tc.tile_pool(name="ep2", bufs=4) as eppool,
            tc.tile_pool(name="ard", bufs=4) as ardpool,
            tc.tile_pool(name="fin", bufs=3) as finpool,
            tc.tile_pool(name="ps2", bufs=2, space="PSUM") as pss,
            tc.tile_pool(name="po2", bufs=2, space="PSUM") as pso,
            tc.tile_pool(name="psz", bufs=2, space="PSUM") as psz,
        ):
            def aread(qc):
                ts = [ardpool.tile([P, E], F16, tag="ard", name="ard")
                      for _ in range(2)]
                for j in range(2):
                    nc.sync.dma_start(ts[j][:], apart[qc * 2 + j][:])
                return ts

            nxt = aread(0)
            for qc in range(NQT):
                ard = nxt
                if qc + 1 < NQT:
                    nxt = aread(qc + 1)
                po = attn_pass(qc, SKH, False, (pss, pso, eppool))

                pz = [psz.tile([P, 1], F32, tag="pz", name="pz")
                      for _ in range(2)]
                for j in range(2):
                    nc.tensor.matmul(pz[j][:], acc[qc][:, j * P:(j + 1) * P],
                                     ones[:], start=True, stop=True)

                for j in range(2):
                    zi = finpool.tile([P, 1], F32, tag="zi", name="zi")
                    nc.vector.reciprocal(zi[:], pz[j][:])
                    r = qc * 2 + j
                    # half-column pipelining: vector add (psum+partial),
                    # scalar copy applying 1/Z, DMA out
                    for oh in range(2):
                        cs = slice(oh * 512, (oh + 1) * 512)
                        ob = finpool.tile([P, 512], F32, tag="ob", name="ob")
                        nc.vector.tensor_add(ob[:], po[j][:, cs],
                                             ard[j][:, cs])
                        ob2 = finpool.tile([P, 512], F32, tag="ob2",
                                           name="ob2")
                        nc.scalar.activation(ob2[:], ob[:], AF.Copy,
                                             scale=zi[:])
                        if with_bv:
                            nc.vector.tensor_add(ob2[:], ob2[:],
                                                 bv_sb[:, cs])
                        nc.gpsimd.dma_start(out[r * P:(r + 1) * P, cs],
                                            ob2[:])
        accpool.release()
        qtpool.release()
        kvp.release()


def _build(with_bv):
    key = ("nc", with_bv)
    if key in _CACHE:
        return _CACHE[key]
    nc = bacc.Bacc("TRN2", target_bir_lowering=False, debug=False,
                   num_devices=N_CORES)
    dram = (
        nc.dram_tensor("xqT", [E, SQ], BF16, kind="ExternalInput"),
        nc.dram_tensor("xkT", [E, SQ], BF16, kind="ExternalInput"),
        nc.dram_tensor("xvT", [E, SQ], BF16, kind="ExternalInput"),
        nc.dram_tensor("wqT", [E, E], BF16, kind="ExternalInput"),
        nc.dram_tensor("wkT", [E, E], BF16, kind="ExternalInput"),
        nc.dram_tensor("wvT", [E, E], BF16, kind="ExternalInput"),
        nc.dram_tensor("bqr", [P, ET], F32, kind="ExternalInput"),
        nc.dram_tensor("bkr", [P, ET], F32, kind="ExternalInput"),
        nc.dram_tensor("bvb", [P, E], F32, kind="ExternalInput"),
        nc.dram_tensor("out", [SQ, E], F32, kind="ExternalOutput"),
    )
    with tile.TileContext(nc) as tc:
        _emit(nc, tc, dram, with_bv)
    nc.compile()
    _CACHE[key] = nc
    return nc


def _prep_in_maps(query, key, value, Wq, bq, Wk, bk, Wv, bv):
    import ml_dtypes
    bf = ml_dtypes.bfloat16
    wqT = np.ascontiguousarray(np.asarray(Wq, np.float32).T.astype(bf))
    wkT = np.ascontiguousarray(np.asarray(Wk, np.float32).T.astype(bf))
    wvT = np.ascontiguousarray(np.asarray(Wv, np.float32).T.astype(bf))
    bqr = np.ascontiguousarray(np.asarray(bq, np.float32).reshape(ET, P).T)
    bkr = np.ascontiguousarray(np.asarray(bk, np.float32).reshape(ET, P).T)
    bvb = np.ascontiguousarray(
        np.broadcast_to(np.asarray(bv, np.float32), (P, E)))
    query = np.asarray(query, np.float32)
    key = np.asarray(key, np.float32)
    value = np.asarray(value, np.float32)
    in_maps = []
    for c in range(N_CORES):
        b, h = divmod(c, HQ)
        sl = slice(h * SQ, (h + 1) * SQ)
        in_maps.append({
            "xqT": np.ascontiguousarray(query[b, sl, :].T.astype(bf)),
            "xkT": np.ascontiguousarray(key[b, sl, :].T.astype(bf)),
            "xvT": np.ascontiguousarray(value[b, sl, :].T.astype(bf)),
            "wqT": wqT, "wkT": wkT, "wvT": wvT,
            "bqr": bqr, "bkr": bkr, "bvb": bvb,
        })
    return in_maps


def kernel(query, key, value, Wq, bq, Wk, bk, Wv, bv, _run_kwargs=None):
    with_bv = bool(np.any(np.asarray(bv, np.float32)))
    nc = _build(with_bv)
    in_maps = _prep_in_maps(query, key, value, Wq, bq, Wk, bk, Wv, bv)
    res = run_bass_kernel_spmd(nc, in_maps, core_ids=list(range(N_CORES)),
                               **(_run_kwargs or {}))
    out = np.empty((B, S, E), np.float32)
    for c in range(N_CORES):
        b, h = divmod(c, HQ)
        out[b, h * SQ:(h + 1) * SQ, :] = res.results[c]["out"]
    if _run_kwargs:
        _CACHE["last_results"] = res
    return out


# revision 23
# speedup vs baseline: 1.2121x; 1.0032x over previous
"""Trainium2 Bass kernel: batched dense attention
   out = softmax((x_q Wq^T + bq)(x_k Wk^T + bk)^T / sqrt(E)) (x_v Wv^T + bv)

Sharding: 8 cores = 4 batches x 2 query-row halves (sequence-parallel on
Q). K/V projections are also split across the pair (each core projects
its own half of K/V) and the halves are exchanged with one in-pair
AllGather, so every core does exactly 1/8 of the total FLOPs.

Key trick: softmax attention is invariant to a permutation of the key
axis, so each core uses its own LOCAL key order [my half; partner half].
The locally projected half is written straight into the resident SBUF
K^T/V tiles (no DRAM round trip), and only the partner block of the
AllGather output is loaded back, addressed with a partition_id-derived
dynamic DRAM offset.

Device scheme (matmul operands bf16 — measured faster than fp16 on HW
for 512-wide moving operands — with fp32 PSUM accumulation):
  - scores are computed TRANSPOSED (keys on partitions) so the exp'd
    probabilities feed the P@V matmul as the stationary operand with no
    on-device transposes (host pre-transposes x^T / W^T and pre-casts).
  - softmax without max-subtraction (scores ~ N(0,1) at this scale;
    exp is safe in fp32): Z = sum_k exp(s) accumulated via a
    ones-vector matmul; 1/Z applied during the PSUM->SBUF output copy.
  - attention is split into two passes so TensorE never waits on the
    exchange: [2a] every query chunk vs the OWN key half (with its Q
    projection computed just-in-time), partial P@V sums drained to DRAM
    in fp16; [2b] every query chunk vs the PARTNER half, combined with
    the 2a partials and normalized by the full softmax denominator.
    Partner K/V is first touched ~160us after the gather lands.
  - DMA completion semaphores rotate GLOBALLY across queues: any DMA
    whose rotation slot lands after a still-pending collective blocks
    until that collective completes. So every latency-critical load
    (projection feeds, wq, the first 4 query-chunk feeds) is scheduled
    BEFORE the gather, the gather + partner loads are pinned to
    sched-0.25 via tile_wait_until, and everything issued after them
    (late xq feeds, partial writes, readbacks, stores) has >100us of
    deadline slack.
  - attention consumes key tiles with the P@V matmuls trailing the
    score/exp pipeline by one key-tile so PSUM handoffs stay off the
    TensorE critical path.
  - bv folded to the end (softmax rows sum to 1 => P@(V+bv) = P@V+bv),
    and skipped entirely when bv == 0 (a per-bias-pattern kernel
    variant is compiled).
"""

import numpy as np

import concourse.bacc as bacc
import concourse.bass as bass
import concourse.mybir as mybir
import concourse.tile as tile
from concourse.bass_utils import run_bass_kernel_spmd

B, S, E = 4, 4096, 1024
N_CORES = 8
HQ = 2              # halves per batch
SQ = S // HQ        # 2048 rows per core (own query rows / own K,V rows)
P = 128
ET = E // P         # 8 embed tiles
SKT = S // P        # 32 key tiles (local order: 0-15 own, 16-31 partner)
SKH = SKT // 2      # 16 key tiles per half
NQ = 256            # attention query-chunk (2 psum out subtiles)
NQT = SQ // NQ      # 8 chunks per core
CH = 512            # projection column chunk
HCH = SQ // CH      # 4 chunks for the K/V half projections
INV_SCALE = 1.0 / float(E) ** 0.5
GROUPS = [[0, 1], [2, 3], [4, 5], [6, 7]]
QPRE = 4            # query-chunk feeds preloaded ahead of the gather

BF16 = mybir.dt.bfloat16
F16 = mybir.dt.float16   # only for the DMA-staged P@V partials
F32 = mybir.dt.float32
AF = mybir.ActivationFunctionType

_CACHE = {}


def _emit(nc, tc, dram, with_bv):
    xqT, xkT, xvT, wqT, wkT, wvT, bqr, bkr, bvb, out = dram

    # partner-block row offset into the AllGather output (dynamic DMA
    # offsets must live in a register on the issuing engine; the partner
    # loads are issued from the gpsimd queue)
    pid_g = nc.gpsimd.partition_id()
    off_g = (1 - (pid_g & 1)) * (2 * E)

    with (
        tc.tile_pool(name="consts", bufs=1) as cpool,
        tc.tile_pool(name="kv", bufs=1) as kvpool,
        tc.tile_pool(name="ccd", bufs=1, space="DRAM") as dpool,
    ):
        bq_sb = cpool.tile([P, ET], F32)
        nc.gpsimd.dma_start(bq_sb[:], bqr[:])
        bk_sb = cpool.tile([P, ET], F32)
        nc.gpsimd.dma_start(bk_sb[:], bkr[:])
        if with_bv:
            bv_sb = cpool.tile([P, E], F32)
            nc.gpsimd.dma_start(bv_sb[:], bvb[:])
        ones = cpool.tile([P, 1], F32)
        nc.gpsimd.memset(ones[:], 1.0)

        # resident OWN K^T half [E, SQ] and V half [SQ, E] (bf16, 8 MB)
        kTo = [kvpool.tile([P, SQ], BF16, tag=f"kTo{et}", name=f"kTo{et}")
               for et in range(ET)]
        vN = [kvpool.tile([P, E], BF16, tag=f"v{sk}", name=f"v{sk}")
              for sk in range(SKH)]

        # one combined K+V exchange buffer: rows [0,E) = K^T half
        # ([E, SQ], natural layout), rows [E,2E) = V half packed two
        # key-tiles per 128-row band ([128, 2E] per band)
        cb_ib = dpool.tile([2 * E, SQ], BF16, tag="cbi", name="cbi")
        cb_ob = dpool.tile([2 * 2 * E, SQ], BF16, tag="cbo", name="cbo")

        # fp16 own-half P@V partial sums, staged through DRAM between
        # passes 2a and 2b (SBUF has no room for 4MB of partials)
        apart = [dpool.tile([P, E], F16, tag=f"ap{r}", name=f"ap{r}")
                 for r in range(2 * NQT)]

        # ---- Phase 1: half K / half V projections ----
        with (
            tc.tile_pool(name="wkv", bufs=16) as wpool,
            tc.tile_pool(name="xk", bufs=14) as xkpool,
            tc.tile_pool(name="xv", bufs=13) as xvpool,
            tc.tile_pool(name="psk", bufs=3, space="PSUM") as psk,
            tc.tile_pool(name="psv", bufs=4, space="PSUM") as psv,
        ):
            wk_sb = [wpool.tile([P, E], BF16, tag="w", name="wk")
                     for _ in range(ET)]
            for dt in range(ET):
                nc.scalar.dma_start(wk_sb[dt][:], wkT[dt * P:(dt + 1) * P, :])

            def feed(src_t, ch, pool):
                cs = slice(ch * CH, (ch + 1) * CH)
                ts = [pool.tile([P, CH], BF16, tag="x", name="xf")
                      for _ in range(ET)]
                for dt in range(ET):
                    nc.sync.dma_start(ts[dt][:], src_t[dt * P:(dt + 1) * P, cs])
                return ts

            feeds = {("k", 0): feed(xkT, 0, xkpool),
                     ("v", 0): feed(xvT, 0, xvpool)}
            wv_sb = [wpool.tile([P, E], BF16, tag="w", name="wv")
                     for _ in range(ET)]
            for dt in range(ET):
                nc.scalar.dma_start(wv_sb[dt][:], wvT[dt * P:(dt + 1) * P, :])
            for ch in range(HCH):
                cs = slice(ch * CH, (ch + 1) * CH)
                # own-half k^T[e, s-chunk] = sum_d WkT[d, e] * xkT[d, s-chunk]
                # written straight into kT columns [0, SQ)
                xs = feeds.pop(("k", ch))
                if ch + 1 < HCH:
                    feeds[("k", ch + 1)] = feed(xkT, ch + 1, xkpool)
                for et in range(ET):
                    ps = psk.tile([P, CH], F32, tag="pk")
                    for dt in range(ET):
                        nc.tensor.matmul(
                            ps[:], wk_sb[dt][:, et * P:(et + 1) * P], xs[dt][:],
                            start=(dt == 0), stop=(dt == ET - 1))
                    nc.vector.tensor_scalar_add(
                        kTo[et][:, cs], ps[:], bk_sb[:, et:et + 1])
                    nc.scalar.dma_start(cb_ib[et * P:(et + 1) * P, cs],
                                        kTo[et][:, cs])

                # own-half v[s-chunk, e] = sum_d xvT[d, s-chunk] * WvT[d, e]
                xv = feeds.pop(("v", ch))
                if ch + 1 < HCH:
                    feeds[("v", ch + 1)] = feed(xvT, ch + 1, xvpool)
                for si in range(CH // P):
                    sk = ch * (CH // P) + si
                    ph = [psv.tile([P, 512], F32, tag="pv", name="pv")
                          for _ in range(2)]
                    for dt in range(ET):
                        for nh in range(2):
                            nc.tensor.matmul(
                                ph[nh][:],
                                xv[dt][:, si * P:(si + 1) * P],
                                wv_sb[dt][:, nh * 512:(nh + 1) * 512],
                                start=(dt == 0), stop=(dt == ET - 1))
                    for nh in range(2):
                        nc.vector.tensor_copy(
                            vN[sk][:, nh * 512:(nh + 1) * 512], ph[nh][:])
                    # V pack: row band E + (sk//2)*P, col block (sk%2)*E
                    nc.scalar.dma_start(
                        cb_ib[E + (sk // 2) * P:E + (sk // 2 + 1) * P,
                              (sk % 2) * E:(sk % 2 + 1) * E],
                        vN[sk][:, :])

        # ---- long-lived attention state (fits in the space the ----
        # ---- phase-1 pools released)                            ----
        kvp = tc.alloc_tile_pool(name="kvp", bufs=1)
        kTp = kvp.tile([P, ET * SQ], BF16, tag="kTp", name="kTp")
        vNp = kvp.tile([P, SKH * E], BF16, tag="vNp", name="vNp")
        qtpool = tc.alloc_tile_pool(name="qt", bufs=1)
        qt = [[qtpool.tile([P, NQ], BF16, tag=f"qt{qc}_{et}",
                           name=f"qt{qc}_{et}")
               for et in range(ET)] for qc in range(NQT)]
        accpool = tc.alloc_tile_pool(name="acc", bufs=1)
        acc = [accpool.tile([P, NQ], F32, tag=f"acc{qc}", name=f"acc{qc}")
               for qc in range(NQT)]
        # wq + xq feeds: released after 2a (2b needs the space).
        # Emitted before the pinned gather so the scheduler gives these
        # loads rotation slots ahead of it (they land by ~120us).
        wqx = tc.alloc_tile_pool(name="wqx", bufs=1)
        wq_sb = [wqx.tile([P, E], BF16, tag="w", name="wq", bufs=8)
                 for _ in range(ET)]
        for dt in range(ET):
            nc.scalar.dma_start(wq_sb[dt][:], wqT[dt * P:(dt + 1) * P, :])

        def qfeed(qc):
            ts = [wqx.tile([P, NQ], BF16, tag="qx", name="xq", bufs=8 * QPRE)
                  for _ in range(ET)]
            for dt in range(ET):
                nc.sync.dma_start(
                    ts[dt][:],
                    xqT[dt * P:(dt + 1) * P, qc * NQ:(qc + 1) * NQ])
            return ts

        xq_pre = {qc: qfeed(qc) for qc in range(QPRE)}

        def kT(et, sk):
            if sk < SKH:
                return kTo[et][:, sk * P:(sk + 1) * P]
            lo = sk - SKH
            return kTp[:, et * SQ + lo * P:et * SQ + (lo + 1) * P]

        def vT(sk, nh):
            if sk < SKH:
                return vN[sk][:, nh * 512:(nh + 1) * 512]
            lo = sk - SKH
            return vNp[:, lo * E + nh * 512:lo * E + (nh + 1) * 512]

        # ---- the exchange: ONE in-pair AllGather + 2 partner loads ----
        # pinned to sched-0.25: after every latency-critical feed above
        # (so the pending collective can't poison their completion
        # semaphores in the global rotation), but ~250us of real time
        # before 2b first touches partner K/V.
        with tc.tile_wait_until(0.25):
            nc.gpsimd.collective_compute(
                "AllGather", mybir.AluOpType.bypass, replica_groups=GROUPS,
                ins=[cb_ib.opt()], outs=[cb_ob.opt()])
            # K: cb_ob rows [off+et*P+p, s] -> kTp[p, et*SQ + s]
            nc.gpsimd.dma_start(
                kTp.rearrange("p (et s) -> p et s", et=ET),
                cb_ob[bass.ds(off_g, E), :].rearrange(
                    "(et p) s -> p et s", et=ET))
            # V: cb_ob rows [off+E+skh*P+p, two*E+e] -> vNp[p, (2*skh+two)*E+e]
            nc.gpsimd.dma_start(
                vNp.rearrange("p (skh two e) -> p skh two e", skh=SKH // 2,
                              two=2),
                cb_ob[bass.ds(off_g + E, E), :].rearrange(
                    "(skh p) (two e) -> p skh two e", skh=SKH // 2, two=2))

        # ---------------- Phase 2: attention ----------------
        def attn_pass(qc, base, first_pass, pools, pre_tail=None):
            pss, pso, eppool = pools
            po = [pso.tile([P, E], F32, tag="po", name="po")
                  for _ in range(2)]

            def emit_pv(ep, sk, first, last):
                for j in range(2):
                    lhs = ep[:, j * P:(j + 1) * P]
                    for nh in range(2):
                        nc.tensor.matmul(
                            po[j][:, nh * 512:(nh + 1) * 512], lhs,
                            vT(sk, nh), start=first, stop=last)

            # scores/exp run one key-tile ahead of the P@V accumulation
            prev = None
            for idx in range(SKH):
                sk = base + idx
                ps = pss.tile([P, NQ], F32, tag="ps")
                for et in range(ET):
                    nc.tensor.matmul(
                        ps[:], kT(et, sk), qt[qc][et][:],
                        start=(et == 0), stop=(et == ET - 1))
                ep = eppool.tile([P, NQ], BF16, tag="ep")
                nc.scalar.activation(ep[:], ps[:], AF.Exp, scale=INV_SCALE)
                if first_pass and idx == 0:
                    nc.vector.tensor_copy(acc[qc][:], ep[:])
                else:
                    nc.vector.tensor_add(acc[qc][:], acc[qc][:], ep[:])
                if prev is not None:
                    emit_pv(*prev, first=(idx == 1), last=False)
                prev = (ep, sk)
            if pre_tail is not None:
                pre_tail()
            emit_pv(*prev, first=False, last=True)
            return po

        # ---- 2a: JIT Q projection + own key half; drain partials ----
        with (
            tc.tile_pool(name="ep", bufs=4) as eppool,
            tc.tile_pool(name="ast", bufs=2) as astpool,
            tc.tile_pool(name="pss", bufs=3, space="PSUM") as pss,
            tc.tile_pool(name="pso", bufs=2, space="PSUM") as pso,
        ):
            for qc in range(NQT):
                xq = xq_pre.pop(qc)
                if qc + QPRE < NQT:
                    xq_pre[qc + QPRE] = qfeed(qc + QPRE)
                for et in range(ET):
                    ps = pss.tile([P, NQ], F32, tag="ps")
                    for dt in range(ET):
                        nc.tensor.matmul(
                            ps[:], wq_sb[dt][:, et * P:(et + 1) * P], xq[dt][:],
                            start=(dt == 0), stop=(dt == ET - 1))
                    nc.vector.tensor_scalar_add(
                        qt[qc][et][:], ps[:], bq_sb[:, et:et + 1])
                po = attn_pass(qc, 0, True, (pss, pso, eppool))
                # partial writes go on the scalar queue: the gather +
                # partner loads own the gpsimd queue, and a queue-mate
                # emitted after them could delay their issue past 2a
                for j in range(2):
                    ast = astpool.tile([P, E], F16, tag="ast", name="ast")
                    nc.vector.tensor_copy(ast[:], po[j][:])
                    nc.scalar.dma_start(apart[qc * 2 + j][:], ast[:])
        wqx.release()

        # ---- 2b: partner key half; combine partials + normalize ----
        with (
            tc.tile_pool(name="ep2", bufs=4) as eppool,
            tc.tile_pool(name="ard", bufs=4) as ardpool,
            tc.tile_pool(name="fin", bufs=3) as finpool,
            tc.tile_pool(name="ps2", bufs=3, space="PSUM") as pss,
            tc.tile_pool(name="po2", bufs=2, space="PSUM") as pso,
            tc.tile_pool(name="psz", bufs=1, space="PSUM") as psz,
        ):
            def aread(qc):
                ts = [ardpool.tile([P, E], F16, tag="ard", name="ard")
                      for _ in range(2)]
                for j in range(2):
                    nc.sync.dma_start(ts[j][:], apart[qc * 2 + j][:])
                return ts

            nxt = aread(0)
            for qc in range(NQT):
                ard = nxt
                if qc + 1 < NQT:
                    nxt = aread(qc + 1)
                # Z reduce + reciprocal hoisted between the last exp/acc
                # and the trailing P@V matmuls, so 1/Z is ready the moment
                # the PSUM output stops accumulating
                zis = []

                def z_tail():
                    for j in range(2):
                        pz = psz.tile([P, 1], F32, tag="pz", name="pz")
                        nc.tensor.matmul(pz[:], acc[qc][:, j * P:(j + 1) * P],
                                         ones[:], start=True, stop=True)
                        zi = finpool.tile([P, 1], F32, tag="zi", name="zi")
                        nc.vector.reciprocal(zi[:], pz[:])
                        zis.append(zi)

                po = attn_pass(qc, SKH, False, (pss, pso, eppool),
                               pre_tail=z_tail)

                for j in range(2):
                    zi = zis[j]
                    r = qc * 2 + j
                    # half-column pipelining: vector add (psum+partial),
                    # scalar copy applying 1/Z, DMA out
                    for oh in range(2):
                        cs = slice(oh * 512, (oh + 1) * 512)
                        ob = finpool.tile([P, 512], F32, tag="ob", name="ob")
                        nc.vector.tensor_add(ob[:], po[j][:, cs],
                                             ard[j][:, cs])
                        ob2 = finpool.tile([P, 512], F32, tag="ob2",
                                           name="ob2")
                        nc.scalar.activation(ob2[:], ob[:], AF.Copy,
                                             scale=zi[:])
                        if with_bv:
                            nc.vector.tensor_add(ob2[:], ob2[:],
                                                 bv_sb[:, cs])
                        nc.gpsimd.dma_start(out[r * P:(r + 1) * P, cs],
                                            ob2[:])
        accpool.release()
        qtpool.release()
        kvp.release()


def _build(with_bv):
    key = ("nc", with_bv)
    if key in _CACHE:
        return _CACHE[key]
    nc = bacc.Bacc("TRN2", target_bir_lowering=False, debug=False,
                   num_devices=N_CORES)
    dram = (
        nc.dram_tensor("xqT", [E, SQ], BF16, kind="ExternalInput"),
        nc.dram_tensor("xkT", [E, SQ], BF16, kind="ExternalInput"),
        nc.dram_tensor("xvT", [E, SQ], BF16, kind="ExternalInput"),
        nc.dram_tensor("wqT", [E, E], BF16, kind="ExternalInput"),
        nc.dram_tensor("wkT", [E, E], BF16, kind="ExternalInput"),
        nc.dram_tensor("wvT", [E, E], BF16, kind="ExternalInput"),
        nc.dram_tensor("bqr", [P, ET], F32, kind="ExternalInput"),
        nc.dram_tensor("bkr", [P, ET], F32, kind="ExternalInput"),
        nc.dram_tensor("bvb", [P, E], F32, kind="ExternalInput"),
        nc.dram_tensor("out", [SQ, E], F32, kind="ExternalOutput"),
    )
    with tile.TileContext(nc) as tc:
        _emit(nc, tc, dram, with_bv)
    nc.compile()
    _CACHE[key] = nc
    return nc


def _prep_in_maps(query, key, value, Wq, bq, Wk, bk, Wv, bv):
    import ml_dtypes
    bf = ml_dtypes.bfloat16
    wqT = np.ascontiguousarray(np.asarray(Wq, np.float32).T.astype(bf))
    wkT = np.ascontiguousarray(np.asarray(Wk, np.float32).T.astype(bf))
    wvT = np.ascontiguousarray(np.asarray(Wv, np.float32).T.astype(bf))
    bqr = np.ascontiguousarray(np.asarray(bq, np.float32).reshape(ET, P).T)
    bkr = np.ascontiguousarray(np.asarray(bk, np.float32).reshape(ET, P).T)
    bvb = np.ascontiguousarray(
        np.broadcast_to(np.asarray(bv, np.float32), (P, E)))
    query = np.asarray(query, np.float32)
    key = np.asarray(key, np.float32)
    value = np.asarray(value, np.float32)
    in_maps = []
    for c in range(N_CORES):
        b, h = divmod(c, HQ)
        sl = slice(h * SQ, (h + 1) * SQ)
        in_maps.append({
            "xqT": np.ascontiguousarray(query[b, sl, :].T.astype(bf)),
            "xkT": np.ascontiguousarray(key[b, sl, :].T.astype(bf)),
            "xvT": np.ascontiguousarray(value[b, sl, :].T.astype(bf)),
            "wqT": wqT, "wkT": wkT, "wvT": wvT,
            "bqr": bqr, "bkr": bkr, "bvb": bvb,
        })
    return in_maps


def kernel(query, key, value, Wq, bq, Wk, bk, Wv, bv, _run_kwargs=None):
    with_bv = bool(np.any(np.asarray(bv, np.float32)))
    nc = _build(with_bv)
    in_maps = _prep_in_maps(query, key, value, Wq, bq, Wk, bk, Wv, bv)
    res = run_bass_kernel_spmd(nc, in_maps, core_ids=list(range(N_CORES)),
                               **(_run_kwargs or {}))
    out = np.empty((B, S, E), np.float32)
    for c in range(N_CORES):
        b, h = divmod(c, HQ)
        out[b, h * SQ:(h + 1) * SQ, :] = res.results[c]["out"]
    if _run_kwargs:
        _CACHE["last_results"] = res
    return out
